# revision 10
# baseline (speedup 1.0000x reference)
"""Cross-attention kernel for 8 TRN2 NeuronCores.

Reference computation (B=2, LT=LS=2048, D=1024, H=16, HD=64):
    q = (x @ Wq)  k = (mem @ Wk)  v = (mem @ Wv)        split into 16 heads
    attn = softmax(q k^T / 8 + pos + mask)
    out  = (attn @ v  concat heads) @ Wo                returns (out, k, v)

Sharding: head-parallel. Core c owns heads {2c, 2c+1} for BOTH batches, so
each position-bias element is loaded once and reused across the batch dim.
Wo is row-sharded per head group; per-core partial outputs are summed on the
host (tensor-parallel unshard).

Device-side layout choices:
  * All matmul contractions need the contraction dim on SBUF partitions, so
    the host pre-transposes x/memory to [D, L] and the position bias to
    [s, t].  Wq is pre-scaled by 1/sqrt(HD).
  * Scores are computed directly in transposed [s, t] layout; after exp they
    are exactly the moving operand PV needs - no 2048x2048 transpose ever
    happens on device.
  * Softmax skips max-subtraction (logits are ~N(0,1); exp is safe in fp32).
    The denominator comes for free: V gets a ones-column appended, so PV's
    row 64 accumulates sum_s exp(S^T[s,t]).
  * Matmuls run as float32r (FP22): full PE rate at N>=512, ~1e-4 rel err.
"""

import numpy as np

import concourse.bass as bass
import concourse.mybir as mybir
import concourse.tile as tile
from concourse.bass_utils import run_bass_kernel_spmd
from concourse.vector_clock import ScopedClock

B, LT, LS, D, H = 2, 2048, 2048, 1024, 16
HD = D // H            # 64
NCORES = 8
HPC = H // NCORES      # heads per core = 2
CW = HPC * HD          # per-core projection width = 128
F32 = mybir.dt.float32
F32R = mybir.dt.float32r
KT = D // 128          # k-tiles per projection = 8
ST = LS // 128         # s-tiles = 16
TQ = LT // 512         # t quarters = 4
TT = LT // 128         # t tiles = 16


class _TC(tile.TileContext):
    """TileContext legalized for walrus's 1-sync-wait-per-instruction limit.

    The pinned walrus build rejects >1 sync wait on any TPB instruction, but
    this Tile version can attach several. Hoist the extra waits onto
    single-wait NoOps issued just before the instruction on the same engine.
    """

    def _split_multi_waits(self, ordered):
        for bb_name, insts in ordered.items():
            out = []
            for inst in insts:
                si = inst.sync_info
                waits = list(si.on_wait) if si and si.on_wait else []
                if len(waits) > 1:
                    for w in waits[:-1]:
                        nop = mybir.InstNoOp(name=f"I-wsplit-{self.nc.next_id()}")
                        nop.engine = inst.engine
                        nop.sync_info = mybir.SyncInfo(on_wait=[w], on_update=[])
                        out.append(nop)
                    si.on_wait = waits[-1:]
                out.append(inst)
            ordered[bb_name] = out
        return ordered

    def _lower_ordered_insts(self, ordered):
        return super()._lower_ordered_insts(self._split_multi_waits(ordered))

    def _drain_and_barrier(self, tick_clock, wait_clock):
        drain_inst = self.nc.sync.drain()
        wait_clock.add_sem_waits(
            drain_inst.ins, ScopedClock({None: tick_clock.global_clock})
        )
        si = drain_inst.ins.sync_info
        waits = list(si.on_wait or [])
        if len(waits) > 1:
            si.on_wait = [waits[0]]
            for w in waits[1:]:
                nop = self.nc.sync.nop()
                nop.ins.sync_info = mybir.SyncInfo(on_wait=[w], on_update=[])
        self.nc.all_engine_barrier()
        assert self.sems is not None
        popped = self.nc._tile_sem_poison_stack.pop()
        assert popped is self._sem_poison
        self.nc.clear_and_free_semaphores(list(self.sems.allocated().values()))
        self.nc.all_engine_barrier()


def _r(ap):
    return ap.bitcast(F32R)


def _build():
    nc = bass.Bass("TRN2", target_bir_lowering=False, debug=False)

    xT = nc.dram_tensor("xT", [B, D, LT], F32R, kind="ExternalInput").ap()
    mT = nc.dram_tensor("mT", [B, D, LS], F32R, kind="ExternalInput").ap()
    biasT = nc.dram_tensor("biasT", [HPC, LS, LT], F32, kind="ExternalInput").ap()
    wq = nc.dram_tensor("wq", [D, CW], F32R, kind="ExternalInput").ap()
    wk = nc.dram_tensor("wk", [D, CW], F32R, kind="ExternalInput").ap()
    wv = nc.dram_tensor("wv", [D, CW], F32R, kind="ExternalInput").ap()
    wo = nc.dram_tensor("wo", [CW, D], F32R, kind="ExternalInput").ap()
    ident_d = nc.dram_tensor("ident", [128, 128], F32R, kind="ExternalInput").ap()
    ones_d = nc.dram_tensor("ones", [128, ST, HPC, 1], F32R, kind="ExternalInput").ap()
    ones1_d = nc.dram_tensor("ones1", [1, HD], F32R, kind="ExternalInput").ap()

    part = nc.dram_tensor("part", [B, LT, D], F32, kind="ExternalOutput").ap()
    kT_out = nc.dram_tensor("kT", [B, CW, LS], F32R, kind="ExternalOutput").ap()
    vT_out = nc.dram_tensor("vT", [B, CW, LS], F32R, kind="ExternalOutput").ap()

    with _TC(nc) as tc, nc.allow_low_precision("fp32r matmul pipeline; verified against fp32 reference"):
        with (
            tc.tile_pool(name="wpool", bufs=1) as wpool,
            tc.tile_pool(name="qkv", bufs=1) as qkv,
            tc.tile_pool(name="small", bufs=4) as small,
        ):
            # ---- resident weights -------------------------------------
            w_sb = {}
            for name, dram in (("wq", wq), ("wk", wk), ("wv", wv)):
                t = wpool.tile([128, KT, CW], F32R, tag=name, name=name)
                nc.sync.dma_start(out=t, in_=dram.rearrange("(kt p) m -> p kt m", p=128))
                w_sb[name] = t
            wo_sb = []
            for hh in range(HPC):
                wt = wpool.tile([HD, D], F32R, tag=f"wo{hh}", name=f"wo{hh}")
                nc.sync.dma_start(out=wt, in_=wo[hh * HD : (hh + 1) * HD, :])
                wo_sb.append(wt)
            ident = wpool.tile([128, 128], F32R, tag="ident", name="ident")
            nc.sync.dma_start(out=ident, in_=ident_d)
            ones1 = wpool.tile([1, HD], F32R, tag="ones1", name="ones1")
            nc.sync.dma_start(out=ones1, in_=ones1_d)

            # ---- persistent activation buffers ------------------------
            qT_sb = [qkv.tile([CW, LT], F32R, tag=f"qT{b}", name=f"qT{b}") for b in range(B)]
            kT_sb = [qkv.tile([CW, LS], F32R, tag=f"kT{b}", name=f"kT{b}") for b in range(B)]
            vT_sb = [qkv.tile([CW, LS], F32R, tag=f"vT{b}", name=f"vT{b}") for b in range(B)]
            # v in natural [s, dh] layout + ones column, per (s-tile, head)
            vnat = [qkv.tile([128, ST, HPC, HD + 1], F32R, tag=f"vn{b}", name=f"vn{b}") for b in range(B)]
            # attention output^T per (b, head): [dh, t]
            atto = [
                [qkv.tile([HD, LT], F32R, tag=f"at{b}_{hh}", name=f"at{b}_{hh}") for hh in range(HPC)]
                for b in range(B)
            ]

            # ================= Phase 1: projections ====================
            with (
                tc.tile_pool(name="xin", bufs=3) as xin,
                tc.tile_pool(name="ps_proj", bufs=2, space="PSUM") as ps_proj,
            ):
                for b in range(B):
                    for src, wname, dst in (
                        (xT, "wq", qT_sb[b]),
                        (mT, "wk", kT_sb[b]),
                        (mT, "wv", vT_sb[b]),
                    ):
                        ps = ps_proj.tile([CW, 2048], F32, tag="proj", name="ps_proj_t")
                        for kt in range(KT):
                            xt = xin.tile([128, 2048], F32R, tag="xt", name="xt")
                            nc.sync.dma_start(
                                out=xt, in_=src[b, kt * 128 : (kt + 1) * 128, :]
                            )
                            for n4 in range(4):
                                sl = slice(n4 * 512, (n4 + 1) * 512)
                                nc.tensor.matmul(
                                    ps[:, sl],
                                    w_sb[wname][:, kt, :],
                                    xt[:, sl],
                                    start=(kt == 0),
                                    stop=(kt == KT - 1),
                                )
                        nc.vector.tensor_copy(out=dst, in_=ps)
                    nc.sync.dma_start(out=kT_out[b], in_=kT_sb[b])
                    nc.sync.dma_start(out=vT_out[b], in_=vT_sb[b])

            # v^T -> v natural via PE transpose, plus the ones column
            with tc.tile_pool(name="ps_tr", bufs=2, space="PSUM") as ps_tr:
                for b in range(B):
                    nc.sync.dma_start(out=vnat[b][:, :, :, HD : HD + 1], in_=ones_d)
                    for st in range(ST):
                        pt = ps_tr.tile([128, 128], F32R, tag="tr", name="pt")
                        nc.tensor.transpose(
                            pt, vT_sb[b][:, st * 128 : (st + 1) * 128], ident
                        )
                        for hh in range(HPC):
                            nc.vector.tensor_copy(
                                out=vnat[b][:, st, hh, 0:HD],
                                in_=pt[:, hh * HD : (hh + 1) * HD],
                            )

            # ================= Phase 2: attention ======================
            with (
                tc.tile_pool(name="biasp", bufs=2) as biasp,
                tc.tile_pool(name="pp", bufs=4) as pp,
                tc.tile_pool(name="ps_s", bufs=2, space="PSUM") as ps_s,
                tc.tile_pool(name="ps_pv", bufs=4, space="PSUM") as ps_pv,
                tc.tile_pool(name="ps_bcp", bufs=2, space="PSUM") as ps_bcp,
            ):
                for hh in range(HPC):
                    hsl = slice(hh * HD, (hh + 1) * HD)
                    for tq in range(TQ):
                        tsl = slice(tq * 512, (tq + 1) * 512)
                        bias_h = [
                            biasp.tile([128, ST // 2, 512], F32, tag="bias", name="bias_t")
                            for _ in range(2)
                        ]
                        for half in range(2):
                            nc.sync.dma_start(
                                out=bias_h[half],
                                in_=biasT[hh]
                                .rearrange("(st p) t -> p st t", p=128)[
                                    :, half * 8 : (half + 1) * 8, tsl
                                ],
                            )
                        pv = [ps_pv.tile([HD + 1, 512], F32, tag="pv", name="pv_t") for _ in range(B)]
                        for st in range(ST):
                            bias_tile = bias_h[st // 8][:, st % 8, :]
                            for b in range(B):
                                s_ps = ps_s.tile([128, 512], F32, tag="s", name="s_ps")
                                nc.tensor.matmul(
                                    s_ps,
                                    kT_sb[b][hsl, st * 128 : (st + 1) * 128],
                                    qT_sb[b][hsl, tsl],
                                    start=True,
                                    stop=True,
                                )
                                p_sb = pp.tile([128, 512], F32R, tag="p", name="p_sb")
                                nc.vector.tensor_add(p_sb, s_ps, bias_tile)
                                nc.scalar.activation(
                                    p_sb, p_sb, mybir.ActivationFunctionType.Exp
                                )
                                nc.tensor.matmul(
                                    pv[b],
                                    vnat[b][:, st, hh, :],
                                    p_sb,
                                    start=(st == 0),
                                    stop=(st == ST - 1),
                                )
                        for b in range(B):
                            # row HD of pv holds the softmax denominators for
                            # this t-chunk; reciprocal, then broadcast across
                            # the 64 dh partitions via a K=1 PE matmul
                            rec = small.tile([1, 512], F32R, tag="rec", name="rec")
                            nc.vector.reciprocal(out=rec, in_=pv[b][HD : HD + 1, :])
                            ps_bc = ps_bcp.tile([HD, 512], F32, tag="bc", name="ps_bc")
                            nc.tensor.matmul(ps_bc, ones1, rec, start=True, stop=True)
                            bc = small.tile([HD, 512], F32, tag="bc", name="bc")
                            nc.scalar.activation(
                                bc, ps_bc, mybir.ActivationFunctionType.Copy
                            )
                            nc.vector.tensor_mul(
                                atto[b][hh][:, tsl], pv[b][0:HD, :], bc
                            )

            # ================= Phase 3: output projection ==============
            with (
                tc.tile_pool(name="op", bufs=3) as op,
                tc.tile_pool(name="ps_o", bufs=2, space="PSUM") as ps_o,
            ):
                for b in range(B):
                    for tt in range(TT):
                        o_ps = ps_o.tile([128, D], F32, tag="o", name="o_ps")
                        for hh in range(HPC):
                            for n2 in range(2):
                                nsl = slice(n2 * 512, (n2 + 1) * 512)
                                nc.tensor.matmul(
                                    o_ps[:, nsl],
                                    atto[b][hh][:, tt * 128 : (tt + 1) * 128],
                                    wo_sb[hh][:, nsl],
                                    start=(hh == 0),
                                    stop=(hh == HPC - 1),
                                )
                        o_sb = op.tile([128, D], F32, tag="os", name="o_sb")
                        nc.vector.tensor_copy(out=o_sb, in_=o_ps)
                        nc.sync.dma_start(
                            out=part[b, tt * 128 : (tt + 1) * 128, :], in_=o_sb
                        )
    return nc


_CACHE = {}


def kernel(x, memory, position_embedding, mask, Wq, Wk, Wv, Wo):
    x = np.asarray(x, np.float32)
    memory = np.asarray(memory, np.float32)
    position_embedding = np.asarray(position_embedding, np.float32)
    mask = np.asarray(mask, np.float32)
    Wq = np.asarray(Wq, np.float32)
    Wk = np.asarray(Wk, np.float32)
    Wv = np.asarray(Wv, np.float32)
    Wo = np.asarray(Wo, np.float32)

    if "nc" not in _CACHE:
        _CACHE["nc"] = _build()
    nc = _CACHE["nc"]

    xT = np.ascontiguousarray(x.transpose(0, 2, 1))
    mT = np.ascontiguousarray(memory.transpose(0, 2, 1))
    scale = np.float32(1.0 / np.sqrt(HD))
    ident = np.eye(128, dtype=np.float32)
    ones = np.ones((128, ST, HPC, 1), np.float32)
    ones1 = np.ones((1, HD), np.float32)

    in_maps = []
    for c in range(NCORES):
        cols = slice(c * CW, (c + 1) * CW)
        bias = position_embedding[0, c * HPC : (c + 1) * HPC] + mask[None]
        in_maps.append(
            {
                "xT": xT,
                "mT": mT,
                "biasT": np.ascontiguousarray(bias.transpose(0, 2, 1)),
                "wq": np.ascontiguousarray(Wq[:, cols]) * scale,
                "wk": np.ascontiguousarray(Wk[:, cols]),
                "wv": np.ascontiguousarray(Wv[:, cols]),
                "wo": np.ascontiguousarray(Wo[cols, :]),
                "ident": ident,
                "ones": ones,
                "ones1": ones1,
            }
        )

    res = run_bass_kernel_spmd(nc, in_maps, list(range(NCORES))).results

    score_proj = np.zeros((B, LT, D), np.float32)
    k = np.empty((B, H, LS, HD), np.float32)
    v = np.empty((B, H, LS, HD), np.float32)
    for c in range(NCORES):
        score_proj += res[c]["part"]
        kc = res[c]["kT"].reshape(B, HPC, HD, LS).transpose(0, 1, 3, 2)
        vc = res[c]["vT"].reshape(B, HPC, HD, LS).transpose(0, 1, 3, 2)
        k[:, c * HPC : (c + 1) * HPC] = kc
        v[:, c * HPC : (c + 1) * HPC] = vc
    return score_proj, k, v


# revision 11
# speedup vs baseline: 2.0259x; 2.0259x over previous
"""Cross-attention kernel for 8 TRN2 NeuronCores.

Reference computation (B=2, LT=LS=2048, D=1024, H=16, HD=64):
    q = (x @ Wq)  k = (mem @ Wk)  v = (mem @ Wv)        split into 16 heads
    attn = softmax(q k^T / 8 + pos + mask)
    out  = (attn @ v  concat heads) @ Wo                returns (out, k, v)

Sharding: head-parallel. Core c owns heads {2c, 2c+1} for BOTH batches, so
each position-bias element is loaded once and reused across the batch dim.
Wo is row-sharded per head group; per-core partial outputs are summed on the
host (tensor-parallel unshard).

Device-side structure:
  * All matmul contractions need the contraction dim on SBUF partitions, so
    the host pre-transposes x/memory to [D, L] (fp16) and ships the position
    bias as exp(bias)^T in [s, t] layout (bf16).  Wq is pre-scaled by
    1/sqrt(HD).
  * Scores are computed directly in transposed [s, t] layout; exp runs on
    the Scalar engine straight out of PSUM, and the bias enters as an
    all-SBUF bf16 multiply (exp(S+b) = exp(S)*exp(b)) so the Vector engine
    runs in its 2x bf16 mode.  The result is exactly the moving operand PV
    needs - no 2048x2048 transpose ever happens on device.
  * Softmax skips max-subtraction (logits are ~N(0,1); exp is safe in fp32).
    The denominator comes for free: V gets a ones-column appended, so PV's
    row 64 accumulates the per-column sums.
  * Projections run fp16 (inputs ~N(0,1): ~1e-3 rel err); QK runs float32r
    (FP22) to keep logits accurate; PV / output projection run bf16.
"""

import ml_dtypes
import numpy as np

import concourse.bass as bass
import concourse.mybir as mybir
import concourse.tile as tile
from concourse.bass_utils import run_bass_kernel_spmd
from concourse.vector_clock import ScopedClock

B, LT, LS, D, H = 2, 2048, 2048, 1024, 16
HD = D // H            # 64
NCORES = 8
HPC = H // NCORES      # heads per core = 2
CW = HPC * HD          # per-core projection width = 128
F32 = mybir.dt.float32
F32R = mybir.dt.float32r
F16 = mybir.dt.float16
BF16 = mybir.dt.bfloat16
KT = D // 128          # k-tiles per projection = 8
ST = LS // 128         # s-tiles = 16
TQ = LT // 512         # t quarters = 4
TT = LT // 128         # t tiles = 16


class _TC(tile.TileContext):
    """TileContext legalized for walrus's 1-sync-wait-per-instruction limit.

    The pinned walrus build rejects >1 sync wait on any TPB instruction, but
    this Tile version can attach several. Hoist the extra waits onto
    single-wait NoOps issued just before the instruction on the same engine.
    """

    def _split_multi_waits(self, ordered):
        for bb_name, insts in ordered.items():
            out = []
            for inst in insts:
                si = inst.sync_info
                waits = list(si.on_wait) if si and si.on_wait else []
                if len(waits) > 1:
                    for w in waits[:-1]:
                        nop = mybir.InstNoOp(name=f"I-wsplit-{self.nc.next_id()}")
                        nop.engine = inst.engine
                        nop.sync_info = mybir.SyncInfo(on_wait=[w], on_update=[])
                        out.append(nop)
                    si.on_wait = waits[-1:]
                out.append(inst)
            ordered[bb_name] = out
        return ordered

    def _lower_ordered_insts(self, ordered):
        return super()._lower_ordered_insts(self._split_multi_waits(ordered))

    def _drain_and_barrier(self, tick_clock, wait_clock):
        drain_inst = self.nc.sync.drain()
        wait_clock.add_sem_waits(
            drain_inst.ins, ScopedClock({None: tick_clock.global_clock})
        )
        si = drain_inst.ins.sync_info
        waits = list(si.on_wait or [])
        if len(waits) > 1:
            si.on_wait = [waits[0]]
            for w in waits[1:]:
                nop = self.nc.sync.nop()
                nop.ins.sync_info = mybir.SyncInfo(on_wait=[w], on_update=[])
        self.nc.all_engine_barrier()
        assert self.sems is not None
        popped = self.nc._tile_sem_poison_stack.pop()
        assert popped is self._sem_poison
        self.nc.clear_and_free_semaphores(list(self.sems.allocated().values()))
        self.nc.all_engine_barrier()


def _build():
    nc = bass.Bass("TRN2", target_bir_lowering=False, debug=False)

    xT = nc.dram_tensor("xT", [B, D, LT], F16, kind="ExternalInput").ap()
    mT = nc.dram_tensor("mT", [B, D, LS], F16, kind="ExternalInput").ap()
    # exp(position_bias + mask), transposed to [s, t]
    ebiasT = nc.dram_tensor("ebiasT", [HPC, LS, LT], BF16, kind="ExternalInput").ap()
    wq = nc.dram_tensor("wq", [D, CW], F16, kind="ExternalInput").ap()
    wk = nc.dram_tensor("wk", [D, CW], F16, kind="ExternalInput").ap()
    wv = nc.dram_tensor("wv", [D, CW], F16, kind="ExternalInput").ap()
    wo = nc.dram_tensor("wo", [CW, D], BF16, kind="ExternalInput").ap()
    ident_d = nc.dram_tensor("ident", [128, 128], F32R, kind="ExternalInput").ap()
    ones_d = nc.dram_tensor("ones", [128, ST, HPC, 1], BF16, kind="ExternalInput").ap()
    ones1_d = nc.dram_tensor("ones1", [1, HD], BF16, kind="ExternalInput").ap()

    part = nc.dram_tensor("part", [B, LT, D], BF16, kind="ExternalOutput").ap()
    kT_out = nc.dram_tensor("kT", [B, CW, LS], F32R, kind="ExternalOutput").ap()
    vT_out = nc.dram_tensor("vT", [B, CW, LS], F32R, kind="ExternalOutput").ap()

    with _TC(nc) as tc, nc.allow_low_precision("mixed fp16/bf16/fp32r pipeline; verified against fp32 reference"):
        with (
            tc.tile_pool(name="wpool", bufs=1) as wpool,
            tc.tile_pool(name="qkv", bufs=1) as qkv,
            tc.tile_pool(name="small", bufs=4) as small,
        ):
            # ---- resident weights -------------------------------------
            w_sb = {}
            for name, dram in (("wq", wq), ("wk", wk), ("wv", wv)):
                t = wpool.tile([128, KT, CW], F16, tag=name, name=name)
                nc.sync.dma_start(out=t, in_=dram.rearrange("(kt p) m -> p kt m", p=128))
                w_sb[name] = t
            wo_sb = []
            for hh in range(HPC):
                wt = wpool.tile([HD, D], BF16, tag=f"wo{hh}", name=f"wo{hh}")
                nc.sync.dma_start(out=wt, in_=wo[hh * HD : (hh + 1) * HD, :])
                wo_sb.append(wt)
            ident = wpool.tile([128, 128], F32R, tag="ident", name="ident")
            nc.sync.dma_start(out=ident, in_=ident_d)
            ones1 = wpool.tile([1, HD], BF16, tag="ones1", name="ones1")
            nc.sync.dma_start(out=ones1, in_=ones1_d)

            # ---- persistent activation buffers ------------------------
            qT_sb = [qkv.tile([CW, LT], F32R, tag=f"qT{b}", name=f"qT{b}") for b in range(B)]
            kT_sb = [qkv.tile([CW, LS], F32R, tag=f"kT{b}", name=f"kT{b}") for b in range(B)]
            vT_sb = [qkv.tile([CW, LS], F32R, tag=f"vT{b}", name=f"vT{b}") for b in range(B)]
            # v in natural [s, dh] layout + ones column, per (s-tile, head)
            vnat = [qkv.tile([128, ST, HPC, HD + 1], BF16, tag=f"vn{b}", name=f"vn{b}") for b in range(B)]
            # attention output^T per (b, head): [dh, t]
            atto = [
                [qkv.tile([HD, LT], BF16, tag=f"at{b}_{hh}", name=f"at{b}_{hh}") for hh in range(HPC)]
                for b in range(B)
            ]

            # ================= Phase 1: projections ====================
            with (
                tc.tile_pool(name="xin", bufs=3) as xin,
                tc.tile_pool(name="ps_proj", bufs=2, space="PSUM") as ps_proj,
            ):
                for b in range(B):
                    for src, wname, dst in (
                        (xT, "wq", qT_sb[b]),
                        (mT, "wk", kT_sb[b]),
                        (mT, "wv", vT_sb[b]),
                    ):
                        ps = ps_proj.tile([CW, 2048], F32, tag="proj", name="ps_proj_t")
                        for kt in range(KT):
                            xt = xin.tile([128, 2048], F16, tag="xt", name="xt")
                            nc.sync.dma_start(
                                out=xt, in_=src[b, kt * 128 : (kt + 1) * 128, :]
                            )
                            for n4 in range(4):
                                sl = slice(n4 * 512, (n4 + 1) * 512)
                                nc.tensor.matmul(
                                    ps[:, sl],
                                    w_sb[wname][:, kt, :],
                                    xt[:, sl],
                                    start=(kt == 0),
                                    stop=(kt == KT - 1),
                                )
                        nc.vector.tensor_copy(out=dst, in_=ps)
                    nc.sync.dma_start(out=kT_out[b], in_=kT_sb[b])
                    nc.sync.dma_start(out=vT_out[b], in_=vT_sb[b])

            # v^T -> v natural via PE transpose, plus the ones column
            with tc.tile_pool(name="ps_tr", bufs=2, space="PSUM") as ps_tr:
                for b in range(B):
                    nc.sync.dma_start(out=vnat[b][:, :, :, HD : HD + 1], in_=ones_d)
                    for st in range(ST):
                        pt = ps_tr.tile([128, 128], F32R, tag="tr", name="pt")
                        nc.tensor.transpose(
                            pt, vT_sb[b][:, st * 128 : (st + 1) * 128], ident
                        )
                        for hh in range(HPC):
                            nc.vector.tensor_copy(
                                out=vnat[b][:, st, hh, 0:HD],
                                in_=pt[:, hh * HD : (hh + 1) * HD],
                            )

            # ================= Phase 2: attention ======================
            with (
                tc.tile_pool(name="biasp", bufs=2) as biasp,
                tc.tile_pool(name="pp", bufs=4) as pp,
                tc.tile_pool(name="ps_s", bufs=2, space="PSUM") as ps_s,
                tc.tile_pool(name="ps_pv", bufs=4, space="PSUM") as ps_pv,
                tc.tile_pool(name="ps_bcp", bufs=2, space="PSUM") as ps_bcp,
            ):
                for hh in range(HPC):
                    hsl = slice(hh * HD, (hh + 1) * HD)
                    for tq in range(TQ):
                        tsl = slice(tq * 512, (tq + 1) * 512)
                        bias_h = [
                            biasp.tile([128, ST // 2, 512], BF16, tag="bias", name="bias_t")
                            for _ in range(2)
                        ]
                        for half in range(2):
                            nc.sync.dma_start(
                                out=bias_h[half],
                                in_=ebiasT[hh]
                                .rearrange("(st p) t -> p st t", p=128)[
                                    :, half * 8 : (half + 1) * 8, tsl
                                ],
                            )
                        pv = [ps_pv.tile([HD + 1, 512], F32, tag="pv", name="pv_t") for _ in range(B)]
                        for st in range(ST):
                            bias_tile = bias_h[st // 8][:, st % 8, :]
                            for b in range(B):
                                s_ps = ps_s.tile([128, 512], F32, tag="s", name="s_ps")
                                nc.tensor.matmul(
                                    s_ps,
                                    kT_sb[b][hsl, st * 128 : (st + 1) * 128],
                                    qT_sb[b][hsl, tsl],
                                    start=True,
                                    stop=True,
                                )
                                p1 = pp.tile([128, 512], BF16, tag="p1", name="p1")
                                nc.scalar.activation(
                                    p1, s_ps, mybir.ActivationFunctionType.Exp
                                )
                                p_sb = pp.tile([128, 512], BF16, tag="p", name="p_sb")
                                nc.vector.tensor_mul(p_sb, p1, bias_tile)
                                nc.tensor.matmul(
                                    pv[b],
                                    vnat[b][:, st, hh, :],
                                    p_sb,
                                    start=(st == 0),
                                    stop=(st == ST - 1),
                                )
                        for b in range(B):
                            # row HD of pv holds the softmax denominators for
                            # this t-chunk; reciprocal, then broadcast across
                            # the 64 dh partitions via a K=1 PE matmul
                            rec = small.tile([1, 512], BF16, tag="rec", name="rec")
                            nc.vector.reciprocal(out=rec, in_=pv[b][HD : HD + 1, :])
                            ps_bc = ps_bcp.tile([HD, 512], F32, tag="bc", name="ps_bc")
                            nc.tensor.matmul(ps_bc, ones1, rec, start=True, stop=True)
                            bc = small.tile([HD, 512], F32, tag="bc", name="bc")
                            nc.scalar.activation(
                                bc, ps_bc, mybir.ActivationFunctionType.Copy
                            )
                            nc.vector.tensor_mul(
                                atto[b][hh][:, tsl], pv[b][0:HD, :], bc
                            )

            # ================= Phase 3: output projection ==============
            with (
                tc.tile_pool(name="op", bufs=3) as op,
                tc.tile_pool(name="ps_o", bufs=2, space="PSUM") as ps_o,
            ):
                for b in range(B):
                    for tt in range(TT):
                        o_ps = ps_o.tile([128, D], F32, tag="o", name="o_ps")
                        for hh in range(HPC):
                            for n2 in range(2):
                                nsl = slice(n2 * 512, (n2 + 1) * 512)
                                nc.tensor.matmul(
                                    o_ps[:, nsl],
                                    atto[b][hh][:, tt * 128 : (tt + 1) * 128],
                                    wo_sb[hh][:, nsl],
                                    start=(hh == 0),
                                    stop=(hh == HPC - 1),
                                )
                        o_sb = op.tile([128, D], BF16, tag="os", name="o_sb")
                        nc.vector.tensor_copy(out=o_sb, in_=o_ps)
                        nc.sync.dma_start(
                            out=part[b, tt * 128 : (tt + 1) * 128, :], in_=o_sb
                        )
    return nc


_CACHE = {}


def _in_maps(x, memory, position_embedding, mask, Wq, Wk, Wv, Wo):
    xT = np.ascontiguousarray(x.transpose(0, 2, 1)).astype(np.float16)
    mT = np.ascontiguousarray(memory.transpose(0, 2, 1)).astype(np.float16)
    scale = np.float32(1.0 / np.sqrt(HD))
    ident = np.eye(128, dtype=np.float32)
    ones = np.ones((128, ST, HPC, 1), ml_dtypes.bfloat16)
    ones1 = np.ones((1, HD), ml_dtypes.bfloat16)

    maps = []
    for c in range(NCORES):
        cols = slice(c * CW, (c + 1) * CW)
        ebias = np.exp(position_embedding[0, c * HPC : (c + 1) * HPC] + mask[None])
        maps.append(
            {
                "xT": xT,
                "mT": mT,
                "ebiasT": np.ascontiguousarray(ebias.transpose(0, 2, 1)).astype(
                    ml_dtypes.bfloat16
                ),
                "wq": (np.ascontiguousarray(Wq[:, cols]) * scale).astype(np.float16),
                "wk": np.ascontiguousarray(Wk[:, cols]).astype(np.float16),
                "wv": np.ascontiguousarray(Wv[:, cols]).astype(np.float16),
                "wo": np.ascontiguousarray(Wo[cols, :]).astype(ml_dtypes.bfloat16),
                "ident": ident,
                "ones": ones,
                "ones1": ones1,
            }
        )
    return maps


def kernel(x, memory, position_embedding, mask, Wq, Wk, Wv, Wo):
    x = np.asarray(x, np.float32)
    memory = np.asarray(memory, np.float32)
    position_embedding = np.asarray(position_embedding, np.float32)
    mask = np.asarray(mask, np.float32)
    Wq = np.asarray(Wq, np.float32)
    Wk = np.asarray(Wk, np.float32)
    Wv = np.asarray(Wv, np.float32)
    Wo = np.asarray(Wo, np.float32)

    if "nc" not in _CACHE:
        _CACHE["nc"] = _build()
    nc = _CACHE["nc"]

    in_maps = _in_maps(x, memory, position_embedding, mask, Wq, Wk, Wv, Wo)
    res = run_bass_kernel_spmd(nc, in_maps, list(range(NCORES))).results

    score_proj = np.zeros((B, LT, D), np.float32)
    k = np.empty((B, H, LS, HD), np.float32)
    v = np.empty((B, H, LS, HD), np.float32)
    for c in range(NCORES):
        score_proj += np.asarray(res[c]["part"], np.float32)
        kc = res[c]["kT"].reshape(B, HPC, HD, LS).transpose(0, 1, 3, 2)
        vc = res[c]["vT"].reshape(B, HPC, HD, LS).transpose(0, 1, 3, 2)
        k[:, c * HPC : (c + 1) * HPC] = kc
        v[:, c * HPC : (c + 1) * HPC] = vc
    return score_proj, k, v
